# revision 17
# baseline (speedup 1.0000x reference)
# Cost-volume concatenation kernel for Trainium2 (Bass/Tile), SPMD over 8 cores.
#
# Problem: left, right: [B=2, H=64, W=256, C=32] f32.
# out[b, d+48, h, w, :32] = left[b,h,w,:]  * valid(w,d)
# out[b, d+48, h, w, 32:] = right[b,h,w-d,:] * valid(w,d),  d in [-48, 48)
# valid(w,d) = 0 <= w-d < W.  Output [2, 96, 64, 256, 64] f32 (~805 MB).
#
# Sharding: disparity axis. Core k handles the 12 levels d in [12k-48, 12k-36).
# The kernel program is identical on every core; all per-core variation lives in
# the DATA:
#   - rpad:  right pre-shifted by the core's base disparity and zero-padded to
#            width TPAD, so the in-kernel shift is j in [0,12) for every core and
#            the zero padding implements the right-half validity masking.
#   - vrep:  a 0/1 validity mask with the same index structure, replicated
#            across the 128 SBUF partitions; out_left = left * vrep_shifted
#            implements the left-half masking.
#
# SBUF layout: partitions = (h, b) — h-major — p = 2*h + b, 128 partitions;
# free dim = (w, c). h-major matters: the output DMA's DRAM access pattern is
# then [h=64, b=2, wc] with outer dim 64, which HWDGE fans out across all 16
# SDMA engines. (A b-major [2, 64, wc] pattern splits over only 2 engines ->
# ~27 GB/s per core; SWDGE spreads by partition but its descriptor ring
# backpressure caps concurrency at ~4 engines for multi-descriptor transfers.)
#
# Per disparity j the kernel assembles interleaved [left|right] rows in SBUF
# (two f32 tensor ops per w-chunk) and streams them out with 4 MB contiguous
# HWDGE DMAs. Per-core traffic: ~13 MB read + ~100 MB write (memory-bound).

import numpy as np

B, H, W, C = 2, 64, 256, 32
MAX_DISP = 48
D2 = 2 * MAX_DISP            # 96 disparity levels
N_CORES = 8
DPC = D2 // N_CORES          # 12 disparities per core
JPAD = DPC - 1               # 11: shift offset so in-kernel shifts are >= 0
TPAD = 272                   # padded t-width (>= W + JPAD = 267)
P = B * H                    # 128 SBUF partitions = (h, b) h-major
WC = W * C                   # 8192
TC = TPAD * C                # 8704
WCHUNK = 128                 # w-columns per output tile / DMA (4 MB per DMA)
F32 = np.float32

_CACHE = {}


def _build_nc():
    import concourse.bacc as bacc
    import concourse.mybir as mybir
    from concourse.tile import TileContext, add_dep_helper

    f32 = mybir.dt.float32
    nc = bacc.Bacc("TRN2", target_bir_lowering=False, debug=False)
    left_t = nc.dram_tensor("left_flat", [P, WC], f32, kind="ExternalInput")
    rpad_t = nc.dram_tensor("rpad", [P, TC], f32, kind="ExternalInput")
    vrep_t = nc.dram_tensor("vrep", [P, TPAD], f32, kind="ExternalInput")
    out_t = nc.dram_tensor("out", [B, DPC, H, W * 2 * C], f32, kind="ExternalOutput")
    # DMA-side view iterating (j, h, b, cols): outer dim 64 for 16-way fan-out.
    out_perm = out_t.ap().rearrange("b j h m -> j h b m")

    with TileContext(nc) as tc:
        with (
            tc.tile_pool(name="ins", bufs=1) as ipool,
            tc.tile_pool(name="outs", bufs=3) as opool,
        ):
            left_sb = ipool.tile([P, WC], f32, tag="left")
            rpad_sb = ipool.tile([P, TC], f32, tag="rpad")
            vrep_sb = ipool.tile([P, TPAD], f32, tag="vrep")
            # Phased input loads: the head (~4.4 MB) drains alone at full read
            # bandwidth so the first output DMA can start ~20us in; the tail
            # halves are gated to drain underneath the first output DMAs
            # (without the gate, all loads round-robin on the shared SDMA
            # engines at packet granularity and the head finishes no earlier
            # than the whole input set). vrep is one mask value per t column
            # (139 KB total) — the mul broadcasts it across the 32 channels
            # with a step-0 inner AP dim.
            SPLIT = 144 * C  # rpad column split: covers t < 144 (wi=0 needs t<140)
            head = [
                nc.sync.dma_start(out=vrep_sb[:], in_=vrep_t[:]),
                nc.sync.dma_start(out=left_sb[:, : WC // 2], in_=left_t[:, : WC // 2]),
                nc.sync.dma_start(out=rpad_sb[:, :SPLIT], in_=rpad_t[:, :SPLIT]),
            ]
            tail = [
                nc.scalar.dma_start(out=left_sb[:, WC // 2 :], in_=left_t[:, WC // 2 :]),
                nc.scalar.dma_start(out=rpad_sb[:, SPLIT:], in_=rpad_t[:, SPLIT:]),
            ]
            for t_ in tail:
                for h_ in head:
                    add_dep_helper(
                        t_.ins, h_.ins,
                        reason="input tail loads drain after head loads",
                    )

            lv = left_sb[:].rearrange("p (w c) -> p w c", c=C)
            rv = rpad_sb[:].rearrange("p (t c) -> p t c", c=C)
            vv = vrep_sb[:]  # [p, t]; broadcast across c inside the mul

            n_out = 0
            for wi in range(0, W, WCHUNK):
                for j in reversed(range(DPC)):
                    ot = opool.tile([P, WCHUNK * 2 * C], f32, tag="ot")
                    ov = ot[:].rearrange("p (w c) -> p w c", c=2 * C)
                    t0 = wi + JPAD - j
                    nc.vector.tensor_mul(
                        out=ov[:, :, 0:C],
                        in0=lv[:, wi : wi + WCHUNK, :],
                        in1=vv[:, t0 : t0 + WCHUNK, None].broadcast_to(
                            [P, WCHUNK, C]
                        ),
                    )
                    nc.vector.tensor_copy(
                        out=ov[:, :, C : 2 * C],
                        in_=rv[:, t0 : t0 + WCHUNK, :],
                    )
                    # Alternate output DMAs across the two HWDGE rings (SP and
                    # ACT) so each ring always has a queued transfer and the
                    # inter-DMA stagger on a single FIFO disappears.
                    dma_eng = nc.sync if n_out % 2 == 0 else nc.scalar
                    dma_eng.dma_start(
                        out=out_perm[j, :, :, wi * 2 * C : (wi + WCHUNK) * 2 * C],
                        in_=ot[:],
                    )
                    n_out += 1
    nc.finalize()
    return nc


def get_nc():
    if "nc" not in _CACHE:
        _CACHE["nc"] = _build_nc()
    return _CACHE["nc"]


def _hb_major(x):
    """[B, H, rest...] -> [128 = (h, b) h-major, prod(rest)] contiguous."""
    return np.ascontiguousarray(x.transpose(1, 0, 2, 3)).reshape(P, -1)


def prep_inputs(left, right):
    """Build the 8 per-core input maps from full left/right."""
    left = np.ascontiguousarray(left, dtype=F32)
    right = np.ascontiguousarray(right, dtype=F32)
    left_flat = _hb_major(left)
    in_maps = []
    for k in range(N_CORES):
        d0 = DPC * k - MAX_DISP
        shift = JPAD + d0        # rpad[..., t, :] = right[..., t - shift, :]
        rpad = np.zeros((B, H, TPAD, C), F32)
        lo, hi = max(0, shift), min(TPAD, shift + W)
        if lo < hi:
            rpad[:, :, lo:hi, :] = right[:, :, lo - shift : hi - shift, :]
        vk = np.zeros(TPAD, F32)
        vk[lo:hi] = 1.0
        vrep = np.ascontiguousarray(np.broadcast_to(vk, (P, TPAD)))
        in_maps.append(
            {"left_flat": left_flat, "rpad": _hb_major(rpad), "vrep": vrep}
        )
    return in_maps


def run(left, right, **kwargs):
    """Run the SPMD kernel; returns (full_output, BassKernelResults)."""
    from concourse.bass_utils import run_bass_kernel_spmd

    nc = get_nc()
    in_maps = prep_inputs(left, right)
    res = run_bass_kernel_spmd(nc, in_maps, core_ids=list(range(N_CORES)), **kwargs)
    full = np.concatenate(
        [r["out"].reshape(B, DPC, H, W, 2 * C) for r in res.results], axis=1
    )
    return full, res


def kernel(left, right):
    full, _ = run(left, right)
    return full


# revision 22
# speedup vs baseline: 1.0102x; 1.0102x over previous
# Cost-volume concatenation kernel for Trainium2 (Bass/Tile), SPMD over 8 cores.
#
# Problem: left, right: [B=2, H=64, W=256, C=32] f32.
# out[b, d+48, h, w, :32] = left[b,h,w,:]  * valid(w,d)
# out[b, d+48, h, w, 32:] = right[b,h,w-d,:] * valid(w,d),  d in [-48, 48)
# valid(w,d) = 0 <= w-d < W.  Output [2, 96, 64, 256, 64] f32 (~805 MB).
#
# Sharding: disparity axis. Core k handles the 12 levels d in [12k-48, 12k-36).
# The kernel program is identical on every core; all per-core variation lives in
# the DATA:
#   - rpad:  right pre-shifted by the core's base disparity and zero-padded to
#            width TPAD, so the in-kernel shift is j in [0,12) for every core and
#            the zero padding implements the right-half validity masking.
#   - vrep:  a 0/1 validity mask with the same index structure, replicated
#            across the 128 SBUF partitions; out_left = left * vrep_shifted
#            implements the left-half masking.
#
# SBUF layout: partitions = (h, b) — h-major — p = 2*h + b, 128 partitions;
# free dim = (w, c). h-major matters: the output DMA's DRAM access pattern is
# then [h=64, b=2, wc] with outer dim 64, which HWDGE fans out across all 16
# SDMA engines. (A b-major [2, 64, wc] pattern splits over only 2 engines ->
# ~27 GB/s per core; SWDGE spreads by partition but its descriptor ring
# backpressure caps concurrency at ~4 engines for multi-descriptor transfers.)
#
# Per disparity j the kernel assembles interleaved [left|right] rows in SBUF
# (two f32 tensor ops per w-chunk) and streams them out with 4 MB contiguous
# HWDGE DMAs. Per-core traffic: ~13 MB read + ~100 MB write (memory-bound).

import numpy as np

B, H, W, C = 2, 64, 256, 32
MAX_DISP = 48
D2 = 2 * MAX_DISP            # 96 disparity levels
N_CORES = 8
DPC = D2 // N_CORES          # 12 disparities per core
JPAD = DPC - 1               # 11: shift offset so in-kernel shifts are >= 0
TPAD = 272                   # padded t-width (>= W + JPAD = 267)
P = B * H                    # 128 SBUF partitions = (h, b) h-major
WC = W * C                   # 8192
TC = TPAD * C                # 8704
WCHUNK = 64                  # w-columns per output tile / DMA (2 MB per DMA)
F32 = np.float32

_CACHE = {}


def _build_nc():
    import concourse.bacc as bacc
    import concourse.mybir as mybir
    from concourse.tile import TileContext, add_dep_helper

    f32 = mybir.dt.float32
    nc = bacc.Bacc("TRN2", target_bir_lowering=False, debug=False)
    left_t = nc.dram_tensor("left_flat", [P, WC], f32, kind="ExternalInput")
    rpad_t = nc.dram_tensor("rpad", [P, TC], f32, kind="ExternalInput")
    vrep_t = nc.dram_tensor("vrep", [P, TPAD], f32, kind="ExternalInput")
    out_t = nc.dram_tensor("out", [B, DPC, H, W * 2 * C], f32, kind="ExternalOutput")
    # DMA-side view iterating (j, h, b, cols): outer dim 64 for 16-way fan-out.
    out_perm = out_t.ap().rearrange("b j h m -> j h b m")

    with TileContext(nc) as tc:
        with (
            tc.tile_pool(name="ins", bufs=1) as ipool,
            tc.tile_pool(name="outs", bufs=4) as opool,
        ):
            left_sb = ipool.tile([P, WC], f32, tag="left")
            rpad_sb = ipool.tile([P, TC], f32, tag="rpad")
            vrep_sb = ipool.tile([P, TPAD], f32, tag="vrep")
            # Phased input loads: the head (~4.4 MB) drains alone at full read
            # bandwidth so the first output DMA can start ~20us in; the tail
            # halves are gated to drain underneath the first output DMAs
            # (without the gate, all loads round-robin on the shared SDMA
            # engines at packet granularity and the head finishes no earlier
            # than the whole input set). vrep is one mask value per t column
            # (139 KB total) — the mul broadcasts it across the 32 channels
            # with a step-0 inner AP dim.
            SPLIT_L = WCHUNK * C  # left head: w < 64 (everything wi=0 needs)
            SPLIT_R = 76 * C      # rpad head: t < 76 (wi=0 outputs read t < 75)
            head = [
                nc.sync.dma_start(out=vrep_sb[:], in_=vrep_t[:]),
                nc.sync.dma_start(out=left_sb[:, :SPLIT_L], in_=left_t[:, :SPLIT_L]),
                nc.sync.dma_start(out=rpad_sb[:, :SPLIT_R], in_=rpad_t[:, :SPLIT_R]),
            ]
            tail = [
                nc.scalar.dma_start(out=left_sb[:, SPLIT_L:], in_=left_t[:, SPLIT_L:]),
                nc.scalar.dma_start(out=rpad_sb[:, SPLIT_R:], in_=rpad_t[:, SPLIT_R:]),
            ]
            for t_ in tail:
                for h_ in head:
                    add_dep_helper(
                        t_.ins, h_.ins,
                        reason="input tail loads drain after head loads",
                    )

            lv = left_sb[:].rearrange("p (w c) -> p w c", c=C)
            rv = rpad_sb[:].rearrange("p (t c) -> p t c", c=C)
            vv = vrep_sb[:]  # [p, t]; broadcast across c inside the mul

            for wi in range(0, W, WCHUNK):
                for j in reversed(range(DPC)):
                    ot = opool.tile([P, WCHUNK * 2 * C], f32, tag="ot")
                    ov = ot[:].rearrange("p (w c) -> p w c", c=2 * C)
                    t0 = wi + JPAD - j
                    nc.vector.tensor_mul(
                        out=ov[:, :, 0:C],
                        in0=lv[:, wi : wi + WCHUNK, :],
                        in1=vv[:, t0 : t0 + WCHUNK, None].broadcast_to(
                            [P, WCHUNK, C]
                        ),
                    )
                    nc.vector.tensor_copy(
                        out=ov[:, :, C : 2 * C],
                        in_=rv[:, t0 : t0 + WCHUNK, :],
                    )
                    nc.sync.dma_start(
                        out=out_perm[j, :, :, wi * 2 * C : (wi + WCHUNK) * 2 * C],
                        in_=ot[:],
                    )
    nc.finalize()
    return nc


def get_nc():
    if "nc" not in _CACHE:
        _CACHE["nc"] = _build_nc()
    return _CACHE["nc"]


def _hb_major(x):
    """[B, H, rest...] -> [128 = (h, b) h-major, prod(rest)] contiguous."""
    return np.ascontiguousarray(x.transpose(1, 0, 2, 3)).reshape(P, -1)


def prep_inputs(left, right):
    """Build the 8 per-core input maps from full left/right."""
    left = np.ascontiguousarray(left, dtype=F32)
    right = np.ascontiguousarray(right, dtype=F32)
    left_flat = _hb_major(left)
    in_maps = []
    for k in range(N_CORES):
        d0 = DPC * k - MAX_DISP
        shift = JPAD + d0        # rpad[..., t, :] = right[..., t - shift, :]
        rpad = np.zeros((B, H, TPAD, C), F32)
        lo, hi = max(0, shift), min(TPAD, shift + W)
        if lo < hi:
            rpad[:, :, lo:hi, :] = right[:, :, lo - shift : hi - shift, :]
        vk = np.zeros(TPAD, F32)
        vk[lo:hi] = 1.0
        vrep = np.ascontiguousarray(np.broadcast_to(vk, (P, TPAD)))
        in_maps.append(
            {"left_flat": left_flat, "rpad": _hb_major(rpad), "vrep": vrep}
        )
    return in_maps


def run(left, right, **kwargs):
    """Run the SPMD kernel; returns (full_output, BassKernelResults)."""
    from concourse.bass_utils import run_bass_kernel_spmd

    nc = get_nc()
    in_maps = prep_inputs(left, right)
    res = run_bass_kernel_spmd(nc, in_maps, core_ids=list(range(N_CORES)), **kwargs)
    full = np.concatenate(
        [r["out"].reshape(B, DPC, H, W, 2 * C) for r in res.results], axis=1
    )
    return full, res


def kernel(left, right):
    full, _ = run(left, right)
    return full
